# revision 10
# baseline (speedup 1.0000x reference)
"""DeepseekV2 MLA attention (weight-absorbed, MQA-style latent) on 8 TRN2 NeuronCores.

Sharding: data-parallel over batch (B=2) x tensor-parallel over heads (4 heads/core).
Each core computes, for its batch element and its 4 heads, the partial o_proj
output out_t = [HID, S] (transposed layout); the host sums the 4 partials per
batch element and transposes back.

Device kernel layout conventions (per core):
  hidden^T [HID, S] streamed from DRAM.  All projections produce "transposed"
  activations with the output feature on partitions:
    ckv^T [c=512, S], k_pe^T [64, S], q_nope^T [128, S], q_pe^T packs [128, S].
  RoPE is evaluated as q_rot = q_a * cos + q_b * sin where q_a / q_b are two
  projections whose weights were pre-permuted (interleave + rotate-half with
  sign folded) on the host, so no cross-partition ops are needed.
  Scores are computed transposed: scores^T[t, s] = ckv^T.T-contract - softmax
  runs max-free (score magnitudes are ~5 std; verified on host), with the row
  sum obtained by a ones-vector matmul, so no PE transposes of attention
  weights are needed.
"""
import sys

for _p in ("/opt/trn_rl_repo", "/root/.axon_site/_ro/trn_rl_repo"):
    if _p not in sys.path:
        sys.path.insert(0, _p)

import numpy as np

B, S, HID = 2, 2048, 2048
H, DN, DR, KVR, DV = 16, 128, 64, 512, 128
THETA, EPS = 10000.0, 1e-6
SCALE = float((DN + DR) ** -0.5)
NCORES, HL = 8, 4  # 2 (batch) x 4 (head groups of 4)
CH = 512           # s-chunk width (= max fp32 moving operand)


def build_nc(s=S, hid=HID):
    import concourse.bacc as bacc
    import concourse.mybir as mybir
    from concourse import tile

    f32 = mybir.dt.float32
    f32r = mybir.dt.float32r
    Exp = mybir.ActivationFunctionType.Exp
    Sqrt = mybir.ActivationFunctionType.Sqrt
    mult = mybir.AluOpType.mult

    def r(ap):
        return ap.bitcast(f32r)

    NCH = s // CH      # s-chunks
    KT = hid // 128    # contraction tiles over HID
    NT = s // 128      # t-tiles

    nc = bacc.Bacc("TRN2", target_bir_lowering=False, debug=False,
                   enable_asserts=False, num_devices=NCORES)

    hid_d = nc.dram_tensor("hid_t", [hid, s], f32r, kind="ExternalInput").ap()
    wq_d = nc.dram_tensor("wq_t", [hid, HL * DN], f32r, kind="ExternalInput").ap()
    wqpe_d = nc.dram_tensor("wqpe_t", [hid, 512], f32r, kind="ExternalInput").ap()
    wkv_d = nc.dram_tensor("wkv_t", [hid, KVR + 2 * DR], f32r, kind="ExternalInput").ap()
    ln_d = nc.dram_tensor("ln_t", [128, 4], f32, kind="ExternalInput").ap()
    kb_d = nc.dram_tensor("kb", [HL, DN, KVR], f32r, kind="ExternalInput").ap()
    vb_d = nc.dram_tensor("vb_t", [HL, KVR, DV], f32r, kind="ExternalInput").ap()
    wo_d = nc.dram_tensor("wo_t", [HL * DV, hid], f32r, kind="ExternalInput").ap()
    cos_d = nc.dram_tensor("cos_p", [128, s], f32, kind="ExternalInput").ap()
    sin_d = nc.dram_tensor("sin_p", [128, s], f32, kind="ExternalInput").ap()
    mask_d = nc.dram_tensor("masks", [128, 4, CH], f32, kind="ExternalInput").ap()
    ident_d = nc.dram_tensor("ident", [128, 128], f32r, kind="ExternalInput").ap()
    onec_d = nc.dram_tensor("ones_c", [128, 1], f32r, kind="ExternalInput").ap()
    oner_d = nc.dram_tensor("ones_r", [1, 128], f32r, kind="ExternalInput").ap()
    out_d = nc.dram_tensor("out_t", [hid, s], f32, kind="ExternalOutput").ap()

    with tile.TileContext(nc) as tc, \
         nc.allow_low_precision(reason="f32r-typed tiles feed fp32r matmuls; psum accum stays fp32"):
        with tc.tile_pool(name="res", bufs=1) as res, \
             tc.tile_pool(name="psp", bufs=8, space="PSUM") as psp:

            def ps_tile(name):
                return psp.tile([128, CH], f32, tag="ps", name=name)

            # resident tiles
            ckvT = [res.tile([128, s], f32r, name=f"ckvT{ci}") for ci in range(4)]
            kper = res.tile([128, s], f32r, name="kper")
            ckvN = [res.tile([128, KVR], f32r, name=f"ckvN{t}") for t in range(NT)]
            kb_sb = res.tile([128, HL, KVR], f32r, name="kb_sb")
            vb_sb = res.tile([128, HL, 4, DV], f32r, name="vb_sb")
            masks = res.tile([128, 4, CH], f32, name="masks_sb")
            ident = res.tile([128, 128], f32r, name="ident_sb")
            onec = res.tile([128, 1], f32r, name="onec_sb")
            oner = res.tile([1, 128], f32r, name="oner_sb")
            ln_sb = res.tile([128, 4], f32, name="ln_sb")
            zb128 = res.tile([128, 1], f32, name="zb128")
            epsb = res.tile([1, 1], f32, name="epsb")
            nc.vector.memset(zb128[:], 0.0)
            nc.vector.memset(epsb[:], EPS)

            nc.sync.dma_start(kb_sb[:], kb_d.rearrange("h d c -> d h c"))
            nc.sync.dma_start(vb_sb[:], vb_d.rearrange("h (ci p) d -> p h ci d", p=128))
            nc.sync.dma_start(masks[:], mask_d)
            nc.sync.dma_start(ident[:], ident_d)
            nc.sync.dma_start(onec[:], onec_d)
            nc.sync.dma_start(oner[:], oner_d)
            nc.sync.dma_start(ln_sb[:], ln_d)

            # ---------------- pass 1: latent KV (ckv^T, ckv_nat, k_pe rot) ----
            with tc.tile_pool(name="p1", bufs=1) as p1:
                wkv_sb = p1.tile([128, KT, KVR + 2 * DR], f32r, name="wkv_sb")
                nc.sync.dma_start(wkv_sb[:], wkv_d.rearrange("(kt p) c -> p kt c", p=128))
                for j in range(NCH):
                    sl = slice(j * CH, (j + 1) * CH)
                    cos1 = p1.tile([128, CH], f32, tag="cos1", bufs=2, name="cos1")
                    sin1 = p1.tile([128, CH], f32, tag="sin1", bufs=2, name="sin1")
                    nc.sync.dma_start(cos1[:], cos_d[:, sl])
                    nc.sync.dma_start(sin1[:], sin_d[:, sl])

                    cps = [ps_tile(f"cps{ci}") for ci in range(4)]
                    ka_ps = ps_tile("ka_ps")
                    kb_ps = ps_tile("kb_ps")
                    for k in range(KT):
                        ht1 = p1.tile([128, CH], f32r, tag="ht1", bufs=4, name="ht1")
                        nc.sync.dma_start(ht1[:], hid_d[k * 128:(k + 1) * 128, sl])
                        st_, sp_ = (k == 0), (k == KT - 1)
                        for ci in range(4):
                            nc.tensor.matmul(cps[ci][:], r(wkv_sb[:, k, ci * 128:(ci + 1) * 128]),
                                             r(ht1[:]), start=st_, stop=sp_)
                        nc.tensor.matmul(ka_ps[0:64, :], r(wkv_sb[:, k, KVR:KVR + 64]),
                                         r(ht1[:]), start=st_, stop=sp_)
                        nc.tensor.matmul(kb_ps[0:64, :], r(wkv_sb[:, k, KVR + 64:KVR + 128]),
                                         r(ht1[:]), start=st_, stop=sp_)

                    # evacuate raw ckv^T to SBUF (one PSUM input per DVE op)
                    c_sb = []
                    for ci in range(4):
                        t = p1.tile([128, CH], f32, tag="c_sb", bufs=4, name=f"c_sb{ci}")
                        nc.vector.tensor_copy(t[:], cps[ci][:])
                        c_sb.append(t)
                    # RMSNorm over c (partition direction) via ones-matmul
                    var_ps = ps_tile("var_ps")
                    for ci in range(4):
                        sqt = p1.tile([128, CH], f32r, tag="sqt", bufs=2, name="sqt")
                        nc.vector.tensor_mul(sqt[:], c_sb[ci][:], c_sb[ci][:])
                        nc.tensor.matmul(var_ps[0:1, :], r(onec[:]), r(sqt[:]),
                                         start=(ci == 0), stop=(ci == 3))
                    sd1 = p1.tile([1, CH], f32, tag="sd1", bufs=2, name="sd1")
                    nc.scalar.activation(sd1[:], var_ps[0:1, :], Sqrt, bias=epsb[:], scale=1.0 / KVR)
                    iv1 = p1.tile([1, CH], f32r, tag="iv1", bufs=2, name="iv1")
                    nc.vector.reciprocal(iv1[:], sd1[:])
                    bc_ps = ps_tile("bc_ps")
                    nc.tensor.matmul(bc_ps[:], r(oner[:]), r(iv1[:]), start=True, stop=True)
                    for ci in range(4):
                        nc.vector.scalar_tensor_tensor(ckvT[ci][:, sl], c_sb[ci][:],
                                                       ln_sb[:, ci:ci + 1], bc_ps[:],
                                                       op0=mult, op1=mult)
                    # k_pe rope: kper = ka*cos + kb*sin  (rows 0:64), then duplicate
                    kr_t = p1.tile([128, CH], f32, tag="kr_t", bufs=2, name="kr_t")
                    nc.vector.tensor_mul(kper[0:64, sl], ka_ps[0:64, :], cos1[0:64, :])
                    nc.vector.tensor_mul(kr_t[0:64, :], kb_ps[0:64, :], sin1[0:64, :])
                    nc.vector.tensor_add(kper[0:64, sl], kper[0:64, sl], kr_t[0:64, :])
                    nc.sync.dma_start(kper[64:128, sl], kper[0:64, sl])

                    # transpose normed ckv^T -> ckv natural [t, c]
                    for ss in range(4):
                        t_i = 4 * j + ss
                        for ci in range(4):
                            tp_ps = ps_tile("tp_ps")
                            nc.tensor.transpose(r(tp_ps[:, 0:128]),
                                                ckvT[ci][:, t_i * 128:(t_i + 1) * 128], ident[:])
                            nc.vector.tensor_copy(ckvN[t_i][:, ci * 128:(ci + 1) * 128],
                                                  tp_ps[:, 0:128])

            # ---------------- pass 2: q proj + attention + o_proj -------------
            with tc.tile_pool(name="p2", bufs=1) as p2:
                for j in range(NCH):
                    sl = slice(j * CH, (j + 1) * CH)
                    cos2 = p2.tile([128, CH], f32, tag="cos2", bufs=1, name="cos2")
                    sin2 = p2.tile([128, CH], f32, tag="sin2", bufs=1, name="sin2")
                    nc.sync.dma_start(cos2[:], cos_d[:, sl])
                    nc.sync.dma_start(sin2[:], sin_d[:, sl])

                    qn_ps = [ps_tile(f"qn_ps{h}") for h in range(HL)]
                    qa_ps = [ps_tile(f"qa_ps{p}") for p in range(2)]
                    qb_ps = [ps_tile(f"qb_ps{p}") for p in range(2)]
                    for k in range(KT):
                        ht2 = p2.tile([128, CH], f32r, tag="ht2", bufs=3, name="ht2")
                        nc.sync.dma_start(ht2[:], hid_d[k * 128:(k + 1) * 128, sl])
                        wq_sb = p2.tile([128, HL * DN], f32r, tag="wq_sb", bufs=3, name="wq_sb")
                        nc.sync.dma_start(wq_sb[:], wq_d[k * 128:(k + 1) * 128, :])
                        wp_sb = p2.tile([128, 512], f32r, tag="wp_sb", bufs=4, name="wp_sb")
                        nc.sync.dma_start(wp_sb[:], wqpe_d[k * 128:(k + 1) * 128, :])
                        st_, sp_ = (k == 0), (k == KT - 1)
                        for h in range(HL):
                            nc.tensor.matmul(qn_ps[h][:], r(wq_sb[:, h * 128:(h + 1) * 128]),
                                             r(ht2[:]), start=st_, stop=sp_)
                        for p in range(2):
                            nc.tensor.matmul(qa_ps[p][:], r(wp_sb[:, p * 128:(p + 1) * 128]),
                                             r(ht2[:]), start=st_, stop=sp_)
                            nc.tensor.matmul(qb_ps[p][:], r(wp_sb[:, 256 + p * 128:256 + (p + 1) * 128]),
                                             r(ht2[:]), start=st_, stop=sp_)

                    # evacuate q_nope, rope q_pe
                    qn_sb = []
                    for h in range(HL):
                        t = p2.tile([128, CH], f32r, tag="qn_sb", bufs=4, name=f"qn_sb{h}")
                        nc.vector.tensor_copy(t[:], qn_ps[h][:])
                        qn_sb.append(t)
                    qpr = []
                    for p in range(2):
                        t = p2.tile([128, CH], f32r, tag="qpr", bufs=4, name=f"qpr{p}")
                        qr_t = p2.tile([128, CH], f32, tag="qr_t", bufs=1, name="qr_t")
                        nc.vector.tensor_mul(t[:], qa_ps[p][:], cos2[:])
                        nc.vector.tensor_mul(qr_t[:], qb_ps[p][:], sin2[:])
                        nc.vector.tensor_add(t[:], t[:], qr_t[:])
                        qpr.append(t)

                    vo_sb = p2.tile([128, HL, CH], f32r, tag="vo_sb", bufs=1, name="vo_sb")
                    for h in range(HL):
                        # q_lat^T[c, s] per head
                        ql_sb = p2.tile([128, 4, CH], f32r, tag="ql_sb", bufs=2, name="ql_sb")
                        for ci in range(4):
                            ql_ps = ps_tile("ql_ps")
                            nc.tensor.matmul(ql_ps[:], r(kb_sb[:, h, ci * 128:(ci + 1) * 128]),
                                             r(qn_sb[h][:]), start=True, stop=True)
                            nc.vector.tensor_copy(ql_sb[:, ci, :], ql_ps[:])

                        hp, hh = h // 2, (h % 2) * 64
                        ol_ps = [ps_tile(f"ol_ps{ci}") for ci in range(4)]
                        rs_ps = ps_tile("rs_ps")
                        # t-tile order: diagonal tiles first (first is full-width,
                        # carries start=True), then the off-diagonal history tiles.
                        tts = list(range(4 * j, 4 * j + 4)) + list(range(0, 4 * j))
                        for idx, t_i in enumerate(tts):
                            kd = t_i - 4 * j
                            st = 0 if (kd < 0 or j == 0) else (0, 128, 256, 256)[kd]
                            first, last = (idx == 0), (idx == len(tts) - 1)
                            sc_ps = ps_tile("sc_ps")
                            for ci in range(4):
                                nc.tensor.matmul(sc_ps[:, st:], r(ckvT[ci][:, t_i * 128:(t_i + 1) * 128]),
                                                 r(ql_sb[:, ci, st:]), start=(ci == 0), stop=False)
                            nc.tensor.matmul(sc_ps[:, st:],
                                             r(kper[hh:hh + 64, t_i * 128:(t_i + 1) * 128]),
                                             r(qpr[hp][hh:hh + 64, st:]), start=False, stop=True)
                            if kd >= 0:
                                nc.vector.tensor_add(sc_ps[:, st:], sc_ps[:, st:], masks[:, kd, st:])
                            ex_sb = p2.tile([128, CH], f32r, tag="ex_sb", bufs=3, name="ex_sb")
                            nc.scalar.activation(ex_sb[:, st:], sc_ps[:, st:], Exp,
                                                 bias=zb128[:], scale=SCALE)
                            for ci in range(4):
                                nc.tensor.matmul(ol_ps[ci][:, st:], r(ckvN[t_i][:, ci * 128:(ci + 1) * 128]),
                                                 r(ex_sb[:, st:]), start=first, stop=last)
                            nc.tensor.matmul(rs_ps[0:1, st:], r(onec[:]), r(ex_sb[:, st:]),
                                             start=first, stop=last)

                        # softmax denominator -> broadcast tile
                        rv_sb = p2.tile([1, CH], f32r, tag="rv_sb", bufs=2, name="rv_sb")
                        nc.vector.reciprocal(rv_sb[:], rs_ps[0:1, :])
                        bc2_ps = ps_tile("bc2_ps")
                        nc.tensor.matmul(bc2_ps[:], r(oner[:]), r(rv_sb[:]), start=True, stop=True)
                        bc2_sb = p2.tile([128, CH], f32, tag="bc2_sb", bufs=2, name="bc2_sb")
                        nc.vector.tensor_copy(bc2_sb[:], bc2_ps[:])
                        ol_sb = p2.tile([128, 4, CH], f32r, tag="ol_sb", bufs=2, name="ol_sb")
                        for ci in range(4):
                            nc.vector.tensor_mul(ol_sb[:, ci, :], ol_ps[ci][:], bc2_sb[:])
                        # v_b expansion
                        vo_ps = ps_tile("vo_ps")
                        for ci in range(4):
                            nc.tensor.matmul(vo_ps[:], r(vb_sb[:, h, ci, :]), r(ol_sb[:, ci, :]),
                                             start=(ci == 0), stop=(ci == 3))
                        nc.vector.tensor_copy(vo_sb[:, h, :], vo_ps[:])

                    # o_proj partial: out^T[hid, s] = sum_h wo^T.T @ v_out^T
                    for htile in range(KT):
                        wo_sb = p2.tile([128, HL, 128], f32r, tag="wo_sb", bufs=2, name="wo_sb")
                        nc.sync.dma_start(wo_sb[:], wo_d[:, htile * 128:(htile + 1) * 128]
                                          .rearrange("(a p) n -> p a n", p=128))
                        oo_ps = ps_tile("oo_ps")
                        for hh2 in range(HL):
                            nc.tensor.matmul(oo_ps[:], r(wo_sb[:, hh2, :]), r(vo_sb[:, hh2, :]),
                                             start=(hh2 == 0), stop=(hh2 == HL - 1))
                        oo_sb = p2.tile([128, CH], f32, tag="oo_sb", bufs=3, name="oo_sb")
                        nc.vector.tensor_copy(oo_sb[:], oo_ps[:])
                        nc.sync.dma_start(out_d[htile * 128:(htile + 1) * 128, sl], oo_sb[:])

    nc.compile()
    return nc


# ---------------------------------------------------------------------------
# host-side input prep / output assembly
# ---------------------------------------------------------------------------
_PERM = np.concatenate([np.arange(0, DR, 2), np.arange(1, DR, 2)])


def _rope_tables(pos, s):
    inv_freq = 1.0 / (THETA ** (np.arange(0, DR, 2, dtype=np.float64) / DR))
    t = pos.astype(np.float64)
    freqs = t[:, None] * inv_freq
    emb = np.concatenate([freqs, freqs], axis=-1)          # [s, DR]
    cosT = np.cos(emb).T.astype(np.float32)                # [DR, s]
    sinT = np.sin(emb).T.astype(np.float32)
    cos_p = np.ascontiguousarray(np.vstack([cosT, cosT]))  # [128, s]
    sin_p = np.ascontiguousarray(np.vstack([sinT, sinT]))
    return cos_p, sin_p


def _masks():
    t = np.arange(128)[:, None]
    c = np.arange(CH)[None, :]
    m = np.zeros((128, 4, CH), np.float32)
    for kd in range(4):
        m[:, kd, :] = np.where(c >= 128 * kd + t, 0.0, -1e30).astype(np.float32)
    return m


def prep_core_inputs(inputs, core, s=S, hid=HID):
    b, g = core // 4, core % 4
    heads = slice(HL * g, HL * (g + 1))
    hs = np.asarray(inputs["hidden_states"], np.float32)[b, :s, :hid]
    m = {"hid_t": np.ascontiguousarray(hs.T)}

    wq = np.asarray(inputs["q_nope_weight"], np.float32).reshape(H, DN, HID)[heads, :, :hid]
    m["wq_t"] = np.ascontiguousarray(wq.transpose(2, 0, 1).reshape(hid, HL * DN))

    wqp = np.asarray(inputs["q_pe_weight"], np.float32).reshape(H, DR, HID)[heads, :, :hid]
    a = wqp[:, _PERM, :]                                   # [4, 64, hid]
    bv = np.concatenate([-a[:, 32:64], a[:, 0:32]], axis=1)
    A = a.reshape(2, 128, hid)
    Bv = bv.reshape(2, 128, hid)
    m["wqpe_t"] = np.ascontiguousarray(
        np.concatenate([A[0], A[1], Bv[0], Bv[1]], axis=0).T)

    wkv = np.asarray(inputs["kv_a_weight"], np.float32)[:, :hid]
    kpe_a = wkv[KVR:][_PERM]
    kpe_b = np.concatenate([-kpe_a[32:], kpe_a[:32]], axis=0)
    m["wkv_t"] = np.ascontiguousarray(np.concatenate([wkv[:KVR], kpe_a, kpe_b], axis=0).T)

    m["ln_t"] = np.ascontiguousarray(
        np.asarray(inputs["kv_a_ln_weight"], np.float32).reshape(4, 128).T)
    m["kb"] = np.ascontiguousarray(np.asarray(inputs["k_b_weight"], np.float32)[heads])
    m["vb_t"] = np.ascontiguousarray(
        np.asarray(inputs["v_b_weight"], np.float32)[heads].transpose(0, 2, 1))
    m["wo_t"] = np.ascontiguousarray(
        np.asarray(inputs["o_weight"], np.float32)[:hid, HL * DV * g:HL * DV * (g + 1)].T)

    pos = np.asarray(inputs["position_ids"]).reshape(-1)[:s]
    cos_p, sin_p = _rope_tables(pos, s)
    m["cos_p"], m["sin_p"] = cos_p, sin_p
    m["masks"] = _masks()
    m["ident"] = np.eye(128, dtype=np.float32)
    m["ones_c"] = np.ones((128, 1), np.float32)
    m["ones_r"] = np.ones((1, 128), np.float32)
    return m


_NC_CACHE = {}


def _get_nc():
    if "nc" not in _NC_CACHE:
        _NC_CACHE["nc"] = build_nc()
    return _NC_CACHE["nc"]


def kernel(**inputs):
    from concourse import bass_utils

    nc = _get_nc()
    in_maps = [prep_core_inputs(inputs, c) for c in range(NCORES)]
    res = bass_utils.run_bass_kernel_spmd(nc, in_maps, core_ids=list(range(NCORES)))
    out = np.empty((B, S, HID), np.float32)
    for b in range(B):
        acc = np.array(res.results[4 * b]["out_t"], np.float32)
        for g in range(1, 4):
            acc += res.results[4 * b + g]["out_t"]
        out[b] = acc.T
    return out


# revision 19
# speedup vs baseline: 143.0365x; 143.0365x over previous
"""DeepseekV2 MLA attention (weight-absorbed, MQA-style latent) on 8 TRN2 NeuronCores.

Sharding: data-parallel over batch (B=2) x tensor-parallel over heads (4 heads/core).
Each core computes, for its batch element and its 4 heads, the partial o_proj
output out_t = [HID, S] (transposed layout); the host sums the 4 partials per
batch element and transposes back.

Device kernel layout conventions (per core):
  hidden^T [HID, S] streamed from DRAM.  All projections produce "transposed"
  activations with the output feature on partitions:
    ckv^T [c=512, S], k_pe^T [64, S], q_nope^T [128, S], q_pe^T packs [128, S].
  RoPE is evaluated as q_rot = q_a * cos + q_b * sin where q_a / q_b are two
  projections whose weights were pre-permuted (interleave + rotate-half with
  sign folded) on the host, so no cross-partition ops are needed.
  Scores are computed transposed: scores^T[t, s] = ckv^T.T-contract - softmax
  runs max-free (score magnitudes are ~5 std; verified on host), with the row
  sum obtained by a ones-vector matmul, so no PE transposes of attention
  weights are needed.
"""
import sys

for _p in ("/opt/trn_rl_repo", "/root/.axon_site/_ro/trn_rl_repo"):
    if _p not in sys.path:
        sys.path.insert(0, _p)

import numpy as np

B, S, HID = 2, 2048, 2048
H, DN, DR, KVR, DV = 16, 128, 64, 512, 128
THETA, EPS = 10000.0, 1e-6
SCALE = float((DN + DR) ** -0.5)
NCORES, HL = 8, 4  # 2 (batch) x 4 (head groups of 4)
CH = 512           # s-chunk width (= max fp32 moving operand)


def build_nc(s=S, hid=HID, reps=1):
    import concourse.bacc as bacc
    import concourse.mybir as mybir
    from concourse import tile

    f32 = mybir.dt.float32
    f32r = mybir.dt.float32r
    Exp = mybir.ActivationFunctionType.Exp
    Sqrt = mybir.ActivationFunctionType.Sqrt
    mult = mybir.AluOpType.mult

    def r(ap):
        return ap.bitcast(f32r)

    NCH = s // CH      # s-chunks
    KT = hid // 128    # contraction tiles over HID
    NT = s // 128      # t-tiles

    nc = bacc.Bacc("TRN2", target_bir_lowering=False, debug=False,
                   enable_asserts=False, num_devices=NCORES)

    hid_d = nc.dram_tensor("hid_t", [hid, s], f32r, kind="ExternalInput").ap()
    wqa_d = nc.dram_tensor("wqall_t", [hid, 1024], f32r, kind="ExternalInput").ap()
    wkv_d = nc.dram_tensor("wkv_t", [hid, KVR + 2 * DR], f32r, kind="ExternalInput").ap()
    ln_d = nc.dram_tensor("ln_t", [128, 4], f32, kind="ExternalInput").ap()
    kb_d = nc.dram_tensor("kb", [HL, DN, KVR], f32r, kind="ExternalInput").ap()
    vb_d = nc.dram_tensor("vb_t", [HL, KVR, DV], f32r, kind="ExternalInput").ap()
    wo_d = nc.dram_tensor("wo_t", [HL * DV, hid], f32r, kind="ExternalInput").ap()
    cos_d = nc.dram_tensor("cos_p", [128, s], f32, kind="ExternalInput").ap()
    sin_d = nc.dram_tensor("sin_p", [128, s], f32, kind="ExternalInput").ap()
    mask_d = nc.dram_tensor("masks", [128, 4, CH], f32, kind="ExternalInput").ap()
    ident_d = nc.dram_tensor("ident", [128, 128], f32r, kind="ExternalInput").ap()
    onec_d = nc.dram_tensor("ones_c", [128, 1], f32r, kind="ExternalInput").ap()
    oner_d = nc.dram_tensor("ones_r", [1, 128], f32r, kind="ExternalInput").ap()
    out_d = nc.dram_tensor("out_t", [hid, s], f32, kind="ExternalOutput").ap()

    with tile.TileContext(nc) as tc, \
         nc.allow_low_precision(reason="f32r-typed tiles feed fp32r matmuls; psum accum stays fp32"):
        with tc.tile_pool(name="res", bufs=1) as res, \
             tc.tile_pool(name="psp", bufs=8, space="PSUM") as psp:

            def ps_tile(name):
                return psp.tile([128, CH], f32, tag="ps", name=name)

            # resident tiles
            ckvT = [res.tile([128, s], f32r, name=f"ckvT{ci}") for ci in range(4)]
            kper = res.tile([128, s], f32r, name="kper")
            ckvN = [res.tile([128, KVR], f32r, name=f"ckvN{t}") for t in range(NT)]
            kb_sb = res.tile([128, HL, KVR], f32r, name="kb_sb")
            vb_sb = res.tile([128, HL, 4, DV], f32r, name="vb_sb")
            masks = res.tile([128, 4, CH], f32, name="masks_sb")
            ident = res.tile([128, 128], f32r, name="ident_sb")
            onec = res.tile([128, 1], f32r, name="onec_sb")
            oner = res.tile([1, 128], f32r, name="oner_sb")
            ln_sb = res.tile([128, 4], f32, name="ln_sb")
            zb128 = res.tile([128, 1], f32, name="zb128")
            epsb = res.tile([1, 1], f32, name="epsb")
            nc.vector.memset(zb128[:], 0.0)
            nc.vector.memset(epsb[:], EPS)

            nc.scalar.dma_start(kb_sb[:], kb_d.rearrange("h d c -> d h c"))
            nc.scalar.dma_start(vb_sb[:], vb_d.rearrange("h (ci p) d -> p h ci d", p=128))
            nc.scalar.dma_start(masks[:], mask_d)
            nc.scalar.dma_start(ident[:], ident_d)
            nc.scalar.dma_start(onec[:], onec_d)
            nc.scalar.dma_start(oner[:], oner_d)
            nc.scalar.dma_start(ln_sb[:], ln_d)

            # ---------------- pass 1: latent KV (ckv^T, ckv_nat, k_pe rot) ----
            for _rep in range(reps):
              with tc.tile_pool(name="p1", bufs=1) as p1:
                wkv_sb = p1.tile([128, KT, KVR + 2 * DR], f32r, name="wkv_sb")
                for k0 in range(KT):
                    nc.sync.dma_start(wkv_sb[:, k0, :], wkv_d[k0 * 128:(k0 + 1) * 128, :])
                for j in range(NCH):
                    sl = slice(j * CH, (j + 1) * CH)
                    cos1 = p1.tile([128, CH], f32, tag="cos1", bufs=2, name="cos1")
                    sin1 = p1.tile([128, CH], f32, tag="sin1", bufs=2, name="sin1")
                    nc.sync.dma_start(cos1[:], cos_d[:, sl])
                    nc.sync.dma_start(sin1[:], sin_d[:, sl])

                    cps = [ps_tile(f"cps{ci}") for ci in range(4)]
                    ka_ps = ps_tile("ka_ps")
                    kb_ps = ps_tile("kb_ps")
                    for kg in range(KT // 2):
                        ht1 = p1.tile([128, 2, CH], f32r, tag="ht1", bufs=2, name="ht1")
                        nc.sync.dma_start(ht1[:], hid_d[kg * 256:(kg + 1) * 256, sl]
                                          .rearrange("(g p) t -> p g t", p=128))
                        for ki in range(2):
                            k = 2 * kg + ki
                            st_, sp_ = (k == 0), (k == KT - 1)
                            for ci in range(4):
                                nc.tensor.matmul(cps[ci][:], r(wkv_sb[:, k, ci * 128:(ci + 1) * 128]),
                                                 r(ht1[:, ki, :]), start=st_, stop=sp_)
                            nc.tensor.matmul(ka_ps[0:64, :], r(wkv_sb[:, k, KVR:KVR + 64]),
                                             r(ht1[:, ki, :]), start=st_, stop=sp_)
                            nc.tensor.matmul(kb_ps[0:64, :], r(wkv_sb[:, k, KVR + 64:KVR + 128]),
                                             r(ht1[:, ki, :]), start=st_, stop=sp_)

                    # evacuate raw ckv^T to SBUF (one PSUM input per DVE op)
                    c_sb = []
                    for ci in range(4):
                        t = p1.tile([128, CH], f32, tag="c_sb", bufs=4, name=f"c_sb{ci}")
                        nc.vector.tensor_copy(t[:], cps[ci][:])
                        c_sb.append(t)
                    # RMSNorm over c (partition direction) via ones-matmul
                    var_ps = ps_tile("var_ps")
                    for ci in range(4):
                        sqt = p1.tile([128, CH], f32r, tag="sqt", bufs=2, name="sqt")
                        nc.vector.tensor_mul(sqt[:], c_sb[ci][:], c_sb[ci][:])
                        nc.tensor.matmul(var_ps[0:1, :], r(onec[:]), r(sqt[:]),
                                         start=(ci == 0), stop=(ci == 3))
                    sd1 = p1.tile([1, CH], f32, tag="sd1", bufs=2, name="sd1")
                    nc.scalar.activation(sd1[:], var_ps[0:1, :], Sqrt, bias=epsb[:], scale=1.0 / KVR)
                    iv1 = p1.tile([1, CH], f32r, tag="iv1", bufs=2, name="iv1")
                    nc.vector.reciprocal(iv1[:], sd1[:])
                    bc_ps = ps_tile("bc_ps")
                    nc.tensor.matmul(bc_ps[:], r(oner[:]), r(iv1[:]), start=True, stop=True)
                    for ci in range(4):
                        nc.vector.scalar_tensor_tensor(ckvT[ci][:, sl], c_sb[ci][:],
                                                       ln_sb[:, ci:ci + 1], bc_ps[:],
                                                       op0=mult, op1=mult)
                    # k_pe rope: kper = ka*cos + kb*sin  (rows 0:64), then duplicate
                    kr_t = p1.tile([128, CH], f32, tag="kr_t", bufs=2, name="kr_t")
                    nc.vector.tensor_mul(kper[0:64, sl], ka_ps[0:64, :], cos1[0:64, :])
                    nc.vector.tensor_mul(kr_t[0:64, :], kb_ps[0:64, :], sin1[0:64, :])
                    nc.vector.tensor_add(kper[0:64, sl], kper[0:64, sl], kr_t[0:64, :])
                    nc.sync.dma_start(kper[64:128, sl], kper[0:64, sl])

                    # transpose normed ckv^T -> ckv natural [t, c]
                    for ss in range(4):
                        t_i = 4 * j + ss
                        for ci in range(4):
                            tp_ps = ps_tile("tp_ps")
                            nc.tensor.transpose(r(tp_ps[:, 0:128]),
                                                ckvT[ci][:, t_i * 128:(t_i + 1) * 128], ident[:])
                            nc.vector.tensor_copy(ckvN[t_i][:, ci * 128:(ci + 1) * 128],
                                                  tp_ps[:, 0:128])

              # ---------------- pass 2: q proj + attention + o_proj -----------
              with tc.tile_pool(name="p2", bufs=1) as p2:
                for j in range(NCH):
                    sl = slice(j * CH, (j + 1) * CH)
                    cos2 = p2.tile([128, CH], f32, tag="cos2", bufs=1, name="cos2")
                    sin2 = p2.tile([128, CH], f32, tag="sin2", bufs=1, name="sin2")
                    nc.sync.dma_start(cos2[:], cos_d[:, sl])
                    nc.sync.dma_start(sin2[:], sin_d[:, sl])

                    qn_ps = [ps_tile(f"qn_ps{h}") for h in range(HL)]
                    qa_ps = [ps_tile(f"qa_ps{p}") for p in range(2)]
                    qb_ps = [ps_tile(f"qb_ps{p}") for p in range(2)]
                    for kg in range(KT // 2):
                        ht2 = p2.tile([128, 2, CH], f32r, tag="ht2", bufs=2, name="ht2")
                        nc.sync.dma_start(ht2[:], hid_d[kg * 256:(kg + 1) * 256, sl]
                                          .rearrange("(g p) t -> p g t", p=128))
                        wq_sb = p2.tile([128, 2, 1024], f32r, tag="wq_sb", bufs=2, name="wq_sb")
                        nc.sync.dma_start(wq_sb[:], wqa_d[kg * 256:(kg + 1) * 256, :]
                                          .rearrange("(g p) n -> p g n", p=128))
                        for ki in range(2):
                            k = 2 * kg + ki
                            st_, sp_ = (k == 0), (k == KT - 1)
                            for h in range(HL):
                                nc.tensor.matmul(qn_ps[h][:], r(wq_sb[:, ki, h * 128:(h + 1) * 128]),
                                                 r(ht2[:, ki, :]), start=st_, stop=sp_)
                            for p in range(2):
                                nc.tensor.matmul(qa_ps[p][:], r(wq_sb[:, ki, 512 + p * 128:512 + (p + 1) * 128]),
                                                 r(ht2[:, ki, :]), start=st_, stop=sp_)
                                nc.tensor.matmul(qb_ps[p][:], r(wq_sb[:, ki, 768 + p * 128:768 + (p + 1) * 128]),
                                                 r(ht2[:, ki, :]), start=st_, stop=sp_)

                    # evacuate q_nope, rope q_pe
                    qn_sb = []
                    for h in range(HL):
                        t = p2.tile([128, CH], f32r, tag="qn_sb", bufs=4, name=f"qn_sb{h}")
                        nc.vector.tensor_copy(t[:], qn_ps[h][:])
                        qn_sb.append(t)
                    qpr = []
                    for p in range(2):
                        t = p2.tile([128, CH], f32r, tag="qpr", bufs=2, name=f"qpr{p}")
                        qr_t = p2.tile([128, CH], f32, tag="qr_t", bufs=1, name="qr_t")
                        nc.vector.tensor_mul(t[:], qa_ps[p][:], cos2[:])
                        nc.vector.tensor_mul(qr_t[:], qb_ps[p][:], sin2[:])
                        nc.vector.tensor_add(t[:], t[:], qr_t[:])
                        qpr.append(t)

                    vo_sb = p2.tile([128, HL, CH], f32r, tag="vo_sb", bufs=1, name="vo_sb")
                    for h in range(HL):
                        # q_lat^T[c, s] per head
                        ql_sb = p2.tile([128, 4, CH], f32r, tag="ql_sb", bufs=2, name="ql_sb")
                        for ci in range(4):
                            ql_ps = ps_tile("ql_ps")
                            nc.tensor.matmul(ql_ps[:], r(kb_sb[:, h, ci * 128:(ci + 1) * 128]),
                                             r(qn_sb[h][:]), start=True, stop=True)
                            nc.vector.tensor_copy(ql_sb[:, ci, :], ql_ps[:])

                        hp, hh = h // 2, (h % 2) * 64
                        ol_ps = [ps_tile(f"ol_ps{ci}") for ci in range(4)]
                        rs_ps = ps_tile("rs_ps")
                        # t-tile order: diagonal tiles first (first is full-width,
                        # carries start=True), then the off-diagonal history tiles.
                        tts = list(range(4 * j, 4 * j + 4)) + list(range(0, 4 * j))

                        def score_exp(idx):
                            t_i = tts[idx]
                            kd = t_i - 4 * j
                            st = 0 if (kd < 0 or j == 0) else (0, 128, 256, 256)[kd]
                            sc_ps = ps_tile("sc_ps")
                            for ci in range(4):
                                nc.tensor.matmul(sc_ps[:, st:], r(ckvT[ci][:, t_i * 128:(t_i + 1) * 128]),
                                                 r(ql_sb[:, ci, st:]), start=(ci == 0), stop=False)
                            nc.tensor.matmul(sc_ps[:, st:],
                                             r(kper[hh:hh + 64, t_i * 128:(t_i + 1) * 128]),
                                             r(qpr[hp][hh:hh + 64, st:]), start=False, stop=True)
                            if kd >= 0:
                                nc.vector.tensor_add(sc_ps[:, st:], sc_ps[:, st:], masks[:, kd, st:])
                            ex_sb = p2.tile([128, CH], f32r, tag="ex_sb", bufs=4, name="ex_sb")
                            nc.scalar.activation(ex_sb[:, st:], sc_ps[:, st:], Exp,
                                                 bias=zb128[:], scale=SCALE)
                            return ex_sb, st

                        def pv(idx, ex_sb, st):
                            t_i = tts[idx]
                            first, last = (idx == 0), (idx == len(tts) - 1)
                            for ci in range(4):
                                nc.tensor.matmul(ol_ps[ci][:, st:], r(ckvN[t_i][:, ci * 128:(ci + 1) * 128]),
                                                 r(ex_sb[:, st:]), start=first, stop=last)
                            nc.tensor.matmul(rs_ps[0:1, st:], r(onec[:]), r(ex_sb[:, st:]),
                                             start=first, stop=last)

                        pend = None
                        for idx in range(len(tts)):
                            cur = (idx,) + score_exp(idx)
                            if pend is not None:
                                pv(*pend)
                            pend = cur
                        pv(*pend)

                        # softmax denominator -> broadcast tile
                        rv_sb = p2.tile([1, CH], f32r, tag="rv_sb", bufs=2, name="rv_sb")
                        nc.vector.reciprocal(rv_sb[:], rs_ps[0:1, :])
                        bc2_ps = ps_tile("bc2_ps")
                        nc.tensor.matmul(bc2_ps[:], r(oner[:]), r(rv_sb[:]), start=True, stop=True)
                        bc2_sb = p2.tile([128, CH], f32, tag="bc2_sb", bufs=2, name="bc2_sb")
                        nc.vector.tensor_copy(bc2_sb[:], bc2_ps[:])
                        ol_sb = p2.tile([128, 4, CH], f32r, tag="ol_sb", bufs=2, name="ol_sb")
                        for ci in range(4):
                            nc.vector.tensor_mul(ol_sb[:, ci, :], ol_ps[ci][:], bc2_sb[:])
                        # v_b expansion
                        vo_ps = ps_tile("vo_ps")
                        for ci in range(4):
                            nc.tensor.matmul(vo_ps[:], r(vb_sb[:, h, ci, :]), r(ol_sb[:, ci, :]),
                                             start=(ci == 0), stop=(ci == 3))
                        nc.vector.tensor_copy(vo_sb[:, h, :], vo_ps[:])

                    # o_proj partial: out^T[hid, s] = sum_h wo^T.T @ v_out^T
                    for htile in range(KT):
                        wo_sb = p2.tile([128, HL, 128], f32r, tag="wo_sb", bufs=3, name="wo_sb")
                        nc.sync.dma_start(wo_sb[:], wo_d[:, htile * 128:(htile + 1) * 128]
                                            .rearrange("(a p) n -> p a n", p=128))
                        oo_ps = ps_tile("oo_ps")
                        for hh2 in range(HL):
                            nc.tensor.matmul(oo_ps[:], r(wo_sb[:, hh2, :]), r(vo_sb[:, hh2, :]),
                                             start=(hh2 == 0), stop=(hh2 == HL - 1))
                        oo_sb = p2.tile([128, CH], f32, tag="oo_sb", bufs=3, name="oo_sb")
                        nc.vector.tensor_copy(oo_sb[:], oo_ps[:])
                        nc.sync.dma_start(out_d[htile * 128:(htile + 1) * 128, sl], oo_sb[:])

    nc.compile()
    return nc


# ---------------------------------------------------------------------------
# host-side input prep / output assembly
# ---------------------------------------------------------------------------
_PERM = np.concatenate([np.arange(0, DR, 2), np.arange(1, DR, 2)])


def _rope_tables(pos, s):
    inv_freq = 1.0 / (THETA ** (np.arange(0, DR, 2, dtype=np.float64) / DR))
    t = pos.astype(np.float64)
    freqs = t[:, None] * inv_freq
    emb = np.concatenate([freqs, freqs], axis=-1)          # [s, DR]
    cosT = np.cos(emb).T.astype(np.float32)                # [DR, s]
    sinT = np.sin(emb).T.astype(np.float32)
    cos_p = np.ascontiguousarray(np.vstack([cosT, cosT]))  # [128, s]
    sin_p = np.ascontiguousarray(np.vstack([sinT, sinT]))
    return cos_p, sin_p


def _masks():
    t = np.arange(128)[:, None]
    c = np.arange(CH)[None, :]
    m = np.zeros((128, 4, CH), np.float32)
    for kd in range(4):
        m[:, kd, :] = np.where(c >= 128 * kd + t, 0.0, -1e30).astype(np.float32)
    return m


def prep_core_inputs(inputs, core, s=S, hid=HID):
    b, g = core // 4, core % 4
    heads = slice(HL * g, HL * (g + 1))
    hs = np.asarray(inputs["hidden_states"], np.float32)[b, :s, :hid]
    m = {"hid_t": np.ascontiguousarray(hs.T)}

    wq = np.asarray(inputs["q_nope_weight"], np.float32).reshape(H, DN, HID)[heads, :, :hid]
    wq_t = wq.transpose(2, 0, 1).reshape(hid, HL * DN)

    wqp = np.asarray(inputs["q_pe_weight"], np.float32).reshape(H, DR, HID)[heads, :, :hid]
    a = wqp[:, _PERM, :]                                   # [4, 64, hid]
    bv = np.concatenate([-a[:, 32:64], a[:, 0:32]], axis=1)
    A = a.reshape(2, 128, hid)
    Bv = bv.reshape(2, 128, hid)
    wqpe_t = np.concatenate([A[0], A[1], Bv[0], Bv[1]], axis=0).T
    m["wqall_t"] = np.ascontiguousarray(np.concatenate([wq_t, wqpe_t], axis=1))

    wkv = np.asarray(inputs["kv_a_weight"], np.float32)[:, :hid]
    kpe_a = wkv[KVR:][_PERM]
    kpe_b = np.concatenate([-kpe_a[32:], kpe_a[:32]], axis=0)
    m["wkv_t"] = np.ascontiguousarray(np.concatenate([wkv[:KVR], kpe_a, kpe_b], axis=0).T)

    m["ln_t"] = np.ascontiguousarray(
        np.asarray(inputs["kv_a_ln_weight"], np.float32).reshape(4, 128).T)
    m["kb"] = np.ascontiguousarray(np.asarray(inputs["k_b_weight"], np.float32)[heads])
    m["vb_t"] = np.ascontiguousarray(
        np.asarray(inputs["v_b_weight"], np.float32)[heads].transpose(0, 2, 1))
    m["wo_t"] = np.ascontiguousarray(
        np.asarray(inputs["o_weight"], np.float32)[:hid, HL * DV * g:HL * DV * (g + 1)].T)

    pos = np.asarray(inputs["position_ids"]).reshape(-1)[:s]
    cos_p, sin_p = _rope_tables(pos, s)
    m["cos_p"], m["sin_p"] = cos_p, sin_p
    m["masks"] = _masks()
    m["ident"] = np.eye(128, dtype=np.float32)
    m["ones_c"] = np.ones((128, 1), np.float32)
    m["ones_r"] = np.ones((1, 128), np.float32)
    return m


_NC_CACHE = {}


def _get_nc():
    if "nc" not in _NC_CACHE:
        _NC_CACHE["nc"] = build_nc()
    return _NC_CACHE["nc"]


def kernel(**inputs):
    from concourse import bass_utils

    nc = _get_nc()
    in_maps = [prep_core_inputs(inputs, c) for c in range(NCORES)]
    res = bass_utils.run_bass_kernel_spmd(nc, in_maps, core_ids=list(range(NCORES)))
    out = np.empty((B, S, HID), np.float32)
    for b in range(B):
        acc = np.array(res.results[4 * b]["out_t"], np.float32)
        for g in range(1, 4):
            acc += res.results[4 * b + g]["out_t"]
        out[b] = acc.T
    return out
